# revision 1
# baseline (speedup 1.0000x reference)
"""Trainium2 Bass kernel for DegradationRectifyNet block (CSWin-style window
attention + LePE depthwise conv + code-conditioned LN/MLP).

Data-parallel over batch B=8 across 8 NeuronCores; one image per core.
On-chip everything is channel-major (C on partitions, tokens on free dim).

v2 performance notes (vs the original baseline):
  - All big matmul moving operands are f32r or bf16 (the fp32 ones-matmul
    LN stats were paying a 4x cycles-per-row penalty).
  - LN variance is accumulated into a whole-image buffer and hit with ONE
    ACT Sqrt per layernorm: Sqrt/Exp/Gelu live in different ACT tables and
    each table switch costs 1.28us, so per-tile sqrt interleaved with
    exp/gelu thrashed tables. Square/Copy/Identity are in every table.
  - 1/C is folded into the ones stationary so the stats matmuls produce
    E[x], E[x^2] directly; 1/sd uses the fast approx reciprocal.
  - Attention internals (q/k/v, exp scores, AV stationary, epilogue
    transposes) are bf16: same PE rate, half the DVE/DMA bytes, and
    transposes drop from 2.0 to 1.0 cycles/row.
  - PSUM is retagged for pipelining: scores rotate through 2x[128,1024]
    banks with per-half exp, so the PE never waits on the ACT engine; the
    MLP hidden reuses the same tag with per-half gelu.
  - PE p-states: the tensor engine only reaches 2.4GHz after ~3us of
    continuous execution, so the whole schedule is organized to keep its
    queue non-empty (the baseline averaged ~1.1GHz).
"""

import numpy as np

import concourse.bass as bass
import concourse.bacc as bacc
import concourse.tile as tile
from concourse import mybir
from concourse import bass_utils
from concourse.masks import make_identity
from contextlib import ExitStack

F32 = mybir.dt.float32
F32R = mybir.dt.float32r
BF16 = mybir.dt.bfloat16
AF = mybir.ActivationFunctionType
OP = mybir.AluOpType

B = 8
C = 128          # channels
H = W = 64
L = H * W        # 4096 tokens
SS = 8           # split size
CB = C // 2      # branch channels = 64
NH = 4           # heads per branch
D = CB // NH     # head dim = 16
HID = 4 * C      # 512
CHZ = 512
NT = 8           # token tiles
TT = 512         # tokens per tile
NWIN = 8         # windows per branch
EPS = 1e-5

INPUT_NAMES = [
    "x", "z", "ln1_g", "ln1_b", "ln2_g", "ln2_b", "Wz1", "Wz2", "Wqkv",
    "Wproj", "bproj", "lepe_w0", "lepe_b0", "lepe_w1", "lepe_b1",
    "W1", "b1", "W2", "b2",
]
INPUT_SHAPES = {
    "x": [C, L], "z": [CHZ],
    "ln1_g": [C], "ln1_b": [C], "ln2_g": [C], "ln2_b": [C],
    "Wz1": [C, CHZ], "Wz2": [C, CHZ], "Wqkv": [3 * C, C],
    "Wproj": [C, C], "bproj": [C],
    "lepe_w0": [CB, 1, 3, 3], "lepe_b0": [CB],
    "lepe_w1": [CB, 1, 3, 3], "lepe_b1": [CB],
    "W1": [HID, C], "b1": [HID], "W2": [C, HID], "b2": [C],
}


def emit(ctx: ExitStack, tc: tile.TileContext, io: dict):
    nc = tc.nc

    consts = ctx.enter_context(tc.tile_pool(name="consts", bufs=1))
    wpool = ctx.enter_context(tc.tile_pool(name="wpool", bufs=1))
    big = ctx.enter_context(tc.tile_pool(name="big", bufs=1))
    h1p = ctx.enter_context(tc.tile_pool(name="h1p", bufs=2))
    msqp = ctx.enter_context(tc.tile_pool(name="msqp", bufs=2))
    padp = ctx.enter_context(tc.tile_pool(name="padp", bufs=2))
    expp = ctx.enter_context(tc.tile_pool(name="expp", bufs=2))
    v4p = ctx.enter_context(tc.tile_pool(name="v4p", bufs=2))
    epip = ctx.enter_context(tc.tile_pool(name="epip", bufs=2))
    kstp = ctx.enter_context(tc.tile_pool(name="kstp", bufs=1))
    gelp = ctx.enter_context(tc.tile_pool(name="gelp", bufs=2))
    outp = ctx.enter_context(tc.tile_pool(name="outp", bufs=2))

    # one PSUM pool, 3 tags totalling exactly 8 banks:
    #   sp   2 x [128,1024] f32 (4 banks): QK score halves / LN stats / MLP hidden
    #   avh  1 x [32,512]   f32 (1 bank):  per-head AV accumulation
    #   lin  3 x [128,512]  f32 (3 banks): linears, weight/epilogue transposes
    # (LePE conv runs on GpSimd, so it needs no PSUM bank.)
    psum = ctx.enter_context(tc.tile_pool(name="psum", bufs=1, space="PSUM"))

    def ps_sp(name):
        return psum.tile([128, 1024], F32, tag="sp", bufs=2, name=name)

    def ps_lin(shape, name, dtype=F32):
        return psum.tile(shape, dtype, tag="lin", bufs=3, name=name)

    dma = nc.sync.dma_start
    mm = nc.tensor.matmul

    # ---------------- input DMAs (x first: LN1 starts on it) --------------
    xT = big.tile([128, L], F32, tag="xT")
    for t in range(NT):
        dma(xT[:, TT * t : TT * t + TT], io["x"][:, TT * t : TT * t + TT])

    # weight staging slab: 20 [128,128] blocks, transposed later on PE.
    # Wz/Wqkv first: the z-conditioned bias columns gate the qkv epilogues.
    wblocks = (
        [("Wz1", io["Wz1"][:, 128 * k : 128 * k + 128]) for k in range(4)]
        + [("Wz2", io["Wz2"][:, 128 * k : 128 * k + 128]) for k in range(4)]
        + [("Wqkv", io["Wqkv"][128 * j : 128 * j + 128, :]) for j in range(3)]
        + [("W1", io["W1"][128 * h : 128 * h + 128, :]) for h in range(4)]
        + [("W2", io["W2"][:, 128 * h : 128 * h + 128]) for h in range(4)]
        + [("Wproj", io["Wproj"][:, :])]
    )
    wst = wpool.tile([128, 128 * len(wblocks)], F32, tag="wst")
    for i, (_, src) in enumerate(wblocks):
        dma(wst[:, 128 * i : 128 * i + 128], src)

    def col(name):
        t = consts.tile([128, 1], F32, tag="col_" + name)
        dma(t[:], io[name].unsqueeze(1))
        return t

    g1c = col("ln1_g")
    bln1 = col("ln1_b")
    g2c = col("ln2_g")
    bln2 = col("ln2_b")
    bprojc = col("bproj")
    b2c = col("b2")

    b1cols = consts.tile([128, 4], F32, tag="b1cols")
    for h in range(4):
        dma(b1cols[:, h : h + 1], io["b1"][128 * h : 128 * h + 128].unsqueeze(1))
    zcols = consts.tile([128, 4], F32, tag="zcols")
    for k in range(4):
        dma(zcols[:, k : k + 1], io["z"][128 * k : 128 * k + 128].unsqueeze(1))

    # lepe bias as a column (rows 0:64 branch0, 64:128 branch1)
    lbias = consts.tile([128, 1], F32, tag="lbias")
    dma(lbias[0:CB, :], io["lepe_b0"].unsqueeze(1))
    dma(lbias[CB:128, :], io["lepe_b1"].unsqueeze(1))

    # conv tap weights: rows 0:64 branch-0 taps (a,b); rows 64:128 branch-1
    # taps transposed (branch-1 window images are stored transposed)
    wcomb = consts.tile([128, 9], F32, tag="wcomb")
    dma(wcomb[0:CB, :], io["lepe_w0"].rearrange("c o a b -> c (o a b)"))
    wtap = consts.tile([64, 9], F32, tag="wtap")
    dma(wtap[:], io["lepe_w1"].rearrange("c o a b -> c (o a b)"))

    # ---------------- constants ----------------
    ident = consts.tile([128, 128], F32, tag="ident")
    make_identity(nc, ident[:])
    identB = consts.tile([128, 128], BF16, tag="identB")
    nc.vector.tensor_copy(identB[:], ident[:])
    # 64x64 identity living at partitions 64:128 (for branch-1 v transposes)
    ident2f = consts.tile([128, 64], F32, tag="ident2f")
    nc.gpsimd.memset(ident2f[:], 0.0)
    nc.gpsimd.affine_select(
        out=ident2f[:], in_=ident2f[:], compare_op=OP.not_equal, fill=1.0,
        base=-64, pattern=[[-1, 64]], channel_multiplier=1,
    )
    ident2 = consts.tile([128, 64], BF16, tag="ident2")
    nc.vector.tensor_copy(ident2[:], ident2f[:])

    onesC = consts.tile([128, 128], F32, tag="onesC")
    nc.gpsimd.memset(onesC[:], 1.0 / C)
    # genuinely-rounded f32r copy: walrus requires f32r matmul operands to be
    # produced by an engine write with f32r output (bitcasts are rejected)
    onesCR = consts.tile([128, 128], F32R, tag="onesCR")
    nc.vector.tensor_copy(onesCR[:], onesC[:])
    epscol = consts.tile([128, 1], F32, tag="epscol")
    nc.gpsimd.memset(epscol[:], EPS)

    nc.gpsimd.tensor_copy(
        wcomb[CB:128, :].rearrange("c (b a) -> c b a", b=3),
        wtap[:].rearrange("c (a b) -> c a b", a=3).transpose([0, 2, 1]),
    )

    # v4 template: 32-wide head slots, col 16 ones (denominator), rest zero
    v4f = consts.tile([128, TT], F32, tag="v4f")
    nc.gpsimd.memset(v4f[:], 0.0)
    nc.vector.memset(
        v4f[:].rearrange("p (s w) -> p s w", s=16)[:, :, 16:17], 1.0
    )
    v4br = []
    for brr in range(2):
        v4 = consts.tile([128, TT], BF16, tag=f"v4br{brr}")
        nc.vector.tensor_copy(v4[:], v4f[:])
        v4br.append(v4)

    # persistent QK stationary slabs: slab h keeps only head h's 16 rows live
    # per branch (rows 64*br+16*h), rest stay zero, so the 64-row contraction
    # reads start at partition base 0/64.
    kst4 = [
        kstp.tile([128, TT], BF16, tag=f"kst{i}", name=f"kst{i}")
        for i in range(NH)
    ]
    for i in range(NH):
        nc.gpsimd.memset(kst4[i][:], 0.0)

    # ---------------- big activation buffers ----------------
    qT = big.tile([128, L], BF16, tag="qT")
    kT = big.tile([128, L], BF16, tag="kT")
    vT = big.tile([128, L], BF16, tag="vT")
    cat = big.tile([128, L], BF16, tag="cat")
    xf2 = big.tile([128, L], F32R, tag="xf2")
    xc4 = big.tile([128, L], F32, tag="xc4")     # x - mean
    var4 = big.tile([128, L], F32, tag="var4")   # variance, then 1/sd
    sd4 = big.tile([128, L], F32, tag="sd4")     # sd scratch

    # ---------------- layernorm: per-tile stats into shared buffers -------
    def ln_stats_tile(src, t):
        # The DMA'd f32 xT cannot feed an f32r matmul (walrus requires a
        # rounding engine write), so its sum goes through an ACT f32r copy.
        sl = slice(TT * t, TT * t + TT)
        xt = src[:, sl]
        xsq = msqp.tile([128, TT], F32R, tag="xsq")
        nc.gpsimd.tensor_mul(xsq[:], xt, xt)
        if xt.dtype == F32:
            xr = msqp.tile([128, TT], F32R, tag="xr")
            nc.scalar.copy(xr[:], xt)
            xmm = xr[:]
        else:
            xmm = xt
        s12 = ps_sp("s12")
        mps = s12[:, 0:TT]
        s2ps = s12[:, TT : 2 * TT]
        mm(mps, onesCR[:], xmm, start=True, stop=True)
        mm(s2ps, onesCR[:], xsq[:], start=True, stop=True)
        msq = msqp.tile([128, TT], F32, tag="msq")
        nc.scalar.activation(msq[:], mps, AF.Square)
        nc.vector.tensor_sub(var4[:, sl], s2ps, msq[:])
        nc.vector.tensor_sub(xc4[:, sl], xt, mps)

    def ln_finish_half(hf):
        # per-half so downstream tiles 0-3 start while 4-7 stats still run;
        # Square sits in every ACT table so this adds no extra table loads
        sl = slice(2048 * hf, 2048 * hf + 2048)
        nc.scalar.activation(sd4[:, sl], var4[:, sl], AF.Sqrt, bias=epscol[:])
        nc.vector.reciprocal_approx_fast(var4[:, sl], sd4[:, sl])  # -> 1/sd

    # ---------------- phase 1: LN1 stats (overlaps x DMA) -----------------
    for t in range(NT):
        ln_stats_tile(xT, t)

    # ---------------- weight transposes (PE busy during LN tail) ---------
    WgT = wpool.tile([128, 3 * C], F32R, tag="WgT")
    WqkvT = wpool.tile([128, 3 * C], F32, tag="WqkvT")
    W1gT = wpool.tile([128, HID], F32R, tag="W1gT")
    W1T = wpool.tile([128, HID], F32, tag="W1T")
    W2T = wpool.tile([128, HID], BF16, tag="W2T")
    WpT = wpool.tile([128, C], BF16, tag="WpT")
    Wz1T = wpool.tile([128, CHZ], F32, tag="Wz1T")
    Wz2T = wpool.tile([128, CHZ], F32, tag="Wz2T")

    def wtrans(i, dst, scale_col=None, copy_dst=None):
        pt = ps_lin([128, 512], "ptw")[:, 0:128]
        nc.tensor.transpose(pt, wst[:, 128 * i : 128 * i + 128], ident[:])
        if scale_col is not None:
            nc.vector.tensor_scalar_mul(dst, pt, scale_col[:])
        else:
            nc.vector.tensor_copy(dst, pt)
        if copy_dst is not None:
            nc.vector.tensor_copy(copy_dst, pt)

    # ---------------- z-conditioned bias columns ----------------
    def zbias(WzT, blnc, tag):
        zp = ps_lin([128, 512], "zp")[:, 0:1]
        for k in range(4):
            mm(
                zp, WzT[:, 128 * k : 128 * k + 128], zcols[:, k : k + 1],
                start=(k == 0), stop=(k == 3),
            )
        bz = consts.tile([128, 1], F32, tag=tag)
        nc.vector.tensor_add(bz[:], zp, blnc[:])
        return bz

    for k in range(4):
        wtrans(k, Wz1T[:, 128 * k : 128 * k + 128])
        wtrans(4 + k, Wz2T[:, 128 * k : 128 * k + 128])
    bz1 = zbias(Wz1T, bln1, "bz1")
    bz2 = zbias(Wz2T, bln2, "bz2")
    for j in range(3):
        wtrans(8 + j, WgT[:, 128 * j : 128 * j + 128], g1c,
               WqkvT[:, 128 * j : 128 * j + 128])
    w0cols = consts.tile([128, 3], F32, tag="w0cols")
    for j in range(3):
        wp = ps_lin([128, 512], "wp")[:, 0:1]
        mm(wp, WqkvT[:, 128 * j : 128 * j + 128], bz1[:], start=True, stop=True)
        nc.vector.tensor_copy(w0cols[:, j : j + 1], wp)

    ln_finish_half(0)  # LN1 tiles 0-3 usable; PE keeps transposing below

    for h in range(4):
        wtrans(11 + h, W1gT[:, 128 * h : 128 * h + 128], g2c,
               W1T[:, 128 * h : 128 * h + 128])
    gbcols = consts.tile([128, 4], F32, tag="gbcols")
    for h in range(4):
        wp = ps_lin([128, 512], "wp")[:, 0:1]
        mm(wp, W1T[:, 128 * h : 128 * h + 128], bz2[:], start=True, stop=True)
        nc.vector.tensor_add(gbcols[:, h : h + 1], wp, b1cols[:, h : h + 1])
    for h in range(4):
        wtrans(15 + h, W2T[:, 128 * h : 128 * h + 128])
    wtrans(19, WpT[:])

    ln_finish_half(1)

    # ---------------- phase 2: qkv projections ----------------
    for t in range(NT):
        sl = slice(TT * t, TT * t + TT)
        hn = h1p.tile([128, TT], F32R, tag="hn")
        nc.vector.tensor_mul(hn[:], xc4[:, sl], var4[:, sl])
        for j, dstT in enumerate((qT, kT, vT)):
            qp = ps_lin([128, 512], "qp")
            mm(qp[:], WgT[:, 128 * j : 128 * j + 128], hn[:],
               start=True, stop=True)
            nc.scalar.activation(
                dstT[:, sl], qp[:], AF.Identity, bias=w0cols[:, j : j + 1]
            )

    # window access patterns -------------------------------------------------
    # branch 0: vertical strip window j = cols [8j,8j+8); raster (h, w)
    # branch 1: horizontal strip window j, stored transposed; raster (w, h)
    def win_ap(src, br, j, p0, psz):
        a3 = src[p0 : p0 + psz, :].rearrange("c (h w) -> c h w", h=H)
        if br == 0:
            return a3[:, :, SS * j : SS * j + SS]
        return a3[:, SS * j : SS * j + SS, :].transpose([0, 2, 1])

    # ---------------- phase 3: attention + lepe, one window pair per j ----
    for j in range(NWIN):
        # LePE: zero-padded q window images (66 x 10), both branches stacked
        pad = padp.tile([128, 660], BF16, tag="pad")
        nc.gpsimd.memset(pad[:], 0.0)
        pad3 = pad[:].rearrange("c (h w) -> c h w", h=66)
        nc.gpsimd.tensor_copy(pad3[0:CB, 1:65, 1:9], win_ap(qT, 0, j, 0, CB))
        nc.gpsimd.tensor_copy(pad3[CB:128, 1:65, 1:9], win_ap(qT, 1, j, CB, CB))
        vst = v4p.tile([128, TT], BF16, tag="vst")
        dma(
            vst[0:CB, :].rearrange("c (a b) -> c a b", a=64),
            win_ap(vT, 0, j, 0, CB),
        )
        dma(vst[CB:128, :], vT[CB:128, TT * j : TT * j + TT])
        # depthwise 3x3 on DVE: per-channel tap weight as a scalar column,
        # shifted window reads, in-place accumulate. Keeps the conv entirely
        # off the (throttle-limited) tensor engine; Pool lacks TensorScalarPtr.
        lepe = padp.tile([128, TT], F32, tag="lepe")
        lepe3 = lepe[:].rearrange("c (h w) -> c h w", h=H)
        taps = [(a, b) for a in (-1, 0, 1) for b in (-1, 0, 1)]
        for idx, (a, b) in enumerate(taps):
            src = pad3[:, 1 + a : 65 + a, 1 + b : 9 + b]
            wcol = wcomb[:, 3 * (a + 1) + (b + 1) : 3 * (a + 1) + (b + 2)]
            if idx == 0:
                nc.vector.tensor_scalar_mul(lepe3, src, wcol)
            else:
                nc.vector.scalar_tensor_tensor(
                    lepe3, src, wcol, lepe3, op0=OP.mult, op1=OP.add
                )

        for br in range(2):
            p0 = CB * br
            # v': token-major v (via PE transpose), into 32-wide head slots
            vps = ps_lin([128, 256], "vps", dtype=BF16)
            idv = identB[0:CB, 0:CB] if br == 0 else ident2[CB:128, :]
            for c in range(4):
                mm(
                    vps[:, 64 * c : 64 * c + 64],
                    vst[p0 : p0 + CB, 128 * c : 128 * c + 128],
                    idv,
                    is_transpose=True,
                    start=(c == 0), stop=(c == 3),
                )
            v4 = v4br[br]
            nc.vector.tensor_copy(
                v4[:].rearrange("p (c h s) -> p c h s", c=4, h=4)[:, :, :, 0:16],
                vps[:].rearrange("p (c h d) -> p c h d", c=4, h=4),
            )

            # fp32r matmuls cannot write PSUM at a partition offset, so each
            # head's AV accumulates at offset 0 and is copied to its A stripe.
            A = epip.tile([128, TT], BF16, tag="A")
            for h in range(NH):
                hp0 = p0 + D * h
                kst = kst4[h]
                if br == 0:
                    dma(
                        kst[hp0 : hp0 + D, :].rearrange("c (a b) -> c a b", a=64),
                        win_ap(kT, 0, j, hp0, D),
                    )
                else:
                    dma(
                        kst[hp0 : hp0 + D, :],
                        kT[hp0 : hp0 + D, TT * j : TT * j + TT],
                    )
                es = expp.tile([128, 4 * TT], BF16, tag="es")
                for half in range(2):
                    sph = ps_sp("sph")
                    for cc in range(2):
                        c = 2 * half + cc
                        mm(
                            sph[:, TT * cc : TT * cc + TT],
                            kst[p0 : p0 + CB, 128 * c : 128 * c + 128],
                            win_ap(qT, br, j, p0, CB),
                            start=True, stop=True,
                        )
                    nc.scalar.activation(
                        es[:, 1024 * half : 1024 * half + 1024],
                        sph[:],
                        AF.Exp,
                        scale=float(D) ** -0.5,
                    )
                avh = psum.tile([32, TT], F32, tag="avh", bufs=1, name="avh")
                for c in range(4):
                    mm(
                        avh[:],
                        v4[:, 128 * c + 32 * h : 128 * c + 32 * h + 32],
                        es[:, TT * c : TT * c + TT],
                        start=(c == 0), stop=(c == 3),
                    )
                nc.vector.tensor_copy(A[32 * h : 32 * h + 32, :], avh[:])

            # epilogue: transpose -> divide by denominators -> transpose back
            Tb = ps_lin([128, TT], "Tb", dtype=BF16)
            for c in range(4):
                mm(
                    Tb[:, 128 * c : 128 * c + 128],
                    A[:, 128 * c : 128 * c + 128],
                    identB[:],
                    is_transpose=True,
                    start=(c == 0), stop=(c == 3),
                )
            Tv = Tb.rearrange("p (c h s) -> p c h s", c=4, h=4)
            R = epip.tile([128, 16], F32, tag="R")
            Rv = R[:].rearrange("p (c h) -> p c h", c=4)
            nc.vector.reciprocal(Rv[:, :, :], Tv[:, :, :, 16])
            E = epip.tile([128, 256], BF16, tag="E")
            Ev = E[:].rearrange("p (c h d) -> p c h d", c=4, h=4)
            nc.vector.tensor_mul(
                Ev[:, :, :, :],
                Tv[:, :, :, 0:16],
                Rv[:, :, :].unsqueeze(3).broadcast_to((128, 4, 4, 16)),
            )
            Ot = ps_lin([CB, TT], "Ot", dtype=BF16)
            for c in range(4):
                mm(
                    Ot[:, 128 * c : 128 * c + 128],
                    E[:, 64 * c : 64 * c + 64],
                    identB[:],
                    is_transpose=True,
                    start=(c == 0), stop=(c == 3),
                )
            # un-window: (attention + lepe_bias) + lepe into cat rows
            nc.vector.scalar_tensor_tensor(
                win_ap(cat, br, j, p0, CB),
                Ot[:].rearrange("c (h w) -> c h w", h=H),
                lbias[p0 : p0 + CB, :],
                lepe[p0 : p0 + CB, :].rearrange("c (h w) -> c h w", h=H),
                op0=OP.add, op1=OP.add,
            )

    # ---------------- phase 4: proj + residual + LN2 stats ----------------
    for t in range(NT):
        sl = slice(TT * t, TT * t + TT)
        ap_ = ps_lin([128, 512], "ap_")
        mm(ap_[:], WpT[:], cat[:, sl], start=True, stop=True)
        nc.vector.scalar_tensor_tensor(
            xf2[:, sl], ap_[:], bprojc[:], xT[:, sl], op0=OP.add, op1=OP.add
        )
        ln_stats_tile(xf2, t)
        if t == 3:
            ln_finish_half(0)  # MLP tiles 0-3 unblocked before proj 4-7 done
    ln_finish_half(1)

    # ---------------- phase 5: MLP + residual ----------------
    for t in range(NT):
        sl = slice(TT * t, TT * t + TT)
        hn = h1p.tile([128, TT], F32R, tag="hn")
        nc.vector.tensor_mul(hn[:], xc4[:, sl], var4[:, sl])
        gel = gelp.tile([128, 4 * TT], BF16, tag="gel")
        for half in range(2):
            hp = ps_sp("hp")
            for hh2 in range(2):
                hh = 2 * half + hh2
                mm(
                    hp[:, TT * hh2 : TT * hh2 + TT],
                    W1gT[:, 128 * hh : 128 * hh + 128],
                    hn[:],
                    start=True, stop=True,
                )
                nc.scalar.activation(
                    gel[:, TT * hh : TT * hh + TT],
                    hp[:, TT * hh2 : TT * hh2 + TT],
                    AF.Gelu,
                    bias=gbcols[:, hh : hh + 1],
                )
        o2 = ps_lin([128, 512], "o2")
        for hh in range(4):
            mm(
                o2[:],
                W2T[:, 128 * hh : 128 * hh + 128],
                gel[:, TT * hh : TT * hh + TT],
                start=(hh == 0), stop=(hh == 3),
            )
        ot = outp.tile([128, TT], F32, tag="ot")
        nc.vector.scalar_tensor_tensor(
            ot[:], o2[:], b2c[:], xf2[:, sl], op0=OP.add, op1=OP.add
        )
        dma(io["out"][:, sl], ot[:])


_NC_CACHE = {}


def build_nc():
    key = "nc"
    if key in _NC_CACHE:
        return _NC_CACHE[key]
    nc = bacc.Bacc("TRN2", target_bir_lowering=False, debug=False)
    io = {}
    for name in INPUT_NAMES:
        io[name] = nc.dram_tensor(
            name, INPUT_SHAPES[name], F32, kind="ExternalInput"
        ).ap()
    io["out"] = nc.dram_tensor("out", [C, L], F32, kind="ExternalOutput").ap()
    with tile.TileContext(nc) as tc:
        with ExitStack() as ctx:
            emit(ctx, tc, io)
    nc.compile()
    _NC_CACHE[key] = nc
    return nc


def make_in_maps(inputs):
    in_maps = []
    for b in range(B):
        m = {
            "x": np.ascontiguousarray(
                inputs["x"][b].reshape(C, L).astype(np.float32)
            ),
            "z": np.ascontiguousarray(inputs["z"][b].astype(np.float32)),
        }
        for name in INPUT_NAMES:
            if name in ("x", "z"):
                continue
            m[name] = np.ascontiguousarray(np.asarray(inputs[name], np.float32))
        in_maps.append(m)
    return in_maps


def kernel(**inputs):
    nc = build_nc()
    in_maps = make_in_maps(inputs)
    res = bass_utils.run_bass_kernel_spmd(nc, in_maps, list(range(B)))
    out = np.stack([res.results[b]["out"].reshape(C, H, W) for b in range(B)])
    return out.astype(np.float32)


if __name__ == "__main__":
    # CoreSim numerics check of core 0 against the reference (dev only).
    import sys

    sys.path.insert(0, "/root/problem")
    import reference

    from concourse.bass_interp import CoreSim

    # CoreSim has no Gelu; patch it (HW has a native erf-gelu table).
    import scipy.special
    from concourse import bass_interp

    _orig_act = bass_interp.InstructionExecutor.visit_InstActivation

    def _patched_act(self, instruction, *, reg_snapshot=None):
        if instruction.func == mybir.ActivationFunctionType.Gelu:
            instruction.func = mybir.ActivationFunctionType.Identity
            try:
                _orig_act(self, instruction, reg_snapshot=reg_snapshot)
            finally:
                instruction.func = mybir.ActivationFunctionType.Gelu
            ov = self.view_ap(
                instruction.outs[0],
                bass_interp.Direction.WRITE,
                instruction,
                reg_snapshot=reg_snapshot,
            )
            x = ov.astype(np.float64)
            ov[:] = (
                x * 0.5 * (1.0 + scipy.special.erf(x / np.sqrt(2.0)))
            ).astype(np.float32)
            return
        return _orig_act(self, instruction, reg_snapshot=reg_snapshot)

    bass_interp.InstructionExecutor.visit_InstActivation = _patched_act

    inputs = {k: np.asarray(v) for k, v in reference.setup_inputs().items()}
    expected = np.asarray(reference.reference(**inputs))

    nc = build_nc()
    print("built+compiled", flush=True)
    sim = CoreSim(nc, require_finite=True, require_nnan=True)
    m = make_in_maps(inputs)[0]
    for k, v in m.items():
        sim.tensor(k)[:] = v
    sim.simulate(check_with_hw=False)
    got = sim.tensor("out").reshape(C, H, W)
    exp0 = expected[0]
    err = np.abs(got - exp0)
    denom = np.abs(exp0).max()
    print("absmax err:", err.max(), "rel:", err.max() / denom)
    print(
        "rms rel:",
        np.sqrt(((got - exp0) ** 2).mean()) / np.sqrt((exp0**2).mean()),
    )



# revision 18
# speedup vs baseline: 1.2370x; 1.2370x over previous
"""Trainium2 Bass kernel for DegradationRectifyNet block (CSWin-style window
attention + LePE depthwise conv + code-conditioned LN/MLP).

Data-parallel over batch B=8 across 8 NeuronCores; one image per core.
On-chip everything is channel-major (C on partitions, tokens on free dim).

v3 restructure (vs v2 461us baseline): the PE was HAM-throttled to 1.2GHz
for 80% of the run and streamed 262k of its 342k rows in per-head QK/AV
matmuls that used only 16 of 128 contraction rows (QK) or 32 of 128
stationary columns (AV).
  - q/k are projected directly into per-branch "head-strip" layouts
    (head h lives at partitions 32h..32h+16, rest zero), so the four
    heads' QK matmuls run CONCURRENTLY as 32-row tile_position row tiles
    (4x fewer PE cycles), reading window tokens via strided APs (the old
    SBUF->SBUF window-gather DMAs disappear).
  - AV is col-tiled: each head's v-stationary writes its own 32-partition
    strip of one accumulating PSUM bank (4x fewer PE cycles), which also
    kills the per-head avh->A copies.
  - Scores live in two [128,1024] head-pair PSUM tags so ACT exp runs as
    [128,1024] instructions while the next chunk's QK fills the other
    pair; ACT-exp is the new critical resource (~16.8M elements).
  - LePE runs once over a zero-separated padded image ([128, 66, 8x10])
    written in-place by the qkv epilogue, so windows need no per-window
    pad/memset and window isolation comes from the zero columns.
"""

import numpy as np

import concourse.bass as bass
import concourse.bacc as bacc
import concourse.tile as tile
from concourse import mybir
from concourse import bass_utils
from concourse.masks import make_identity
from contextlib import ExitStack

F32 = mybir.dt.float32
F32R = mybir.dt.float32r
BF16 = mybir.dt.bfloat16
AF = mybir.ActivationFunctionType
OP = mybir.AluOpType

B = 8
C = 128          # channels
H = W = 64
L = H * W        # 4096 tokens
SS = 8           # split size
CB = C // 2      # branch channels = 64
NH = 4           # heads per branch
D = CB // NH     # head dim = 16
HID = 4 * C      # 512
CHZ = 512
NT = 8           # token tiles
TT = 512         # tokens per tile
NWIN = 8         # windows per branch
EPS = 1e-5

INPUT_NAMES = [
    "x", "z", "ln1_g", "ln1_b", "ln2_g", "ln2_b", "Wz1", "Wz2", "Wqkv",
    "Wproj", "bproj", "lepe_w0", "lepe_b0", "lepe_w1", "lepe_b1",
    "W1", "b1", "W2", "b2",
]
INPUT_SHAPES = {
    "x": [C, L], "z": [CHZ],
    "ln1_g": [C], "ln1_b": [C], "ln2_g": [C], "ln2_b": [C],
    "Wz1": [C, CHZ], "Wz2": [C, CHZ], "Wqkv": [3 * C, C],
    "Wproj": [C, C], "bproj": [C],
    "lepe_w0": [CB, 1, 3, 3], "lepe_b0": [CB],
    "lepe_w1": [CB, 1, 3, 3], "lepe_b1": [CB],
    "W1": [HID, C], "b1": [HID], "W2": [C, HID], "b2": [C],
}


def emit(ctx: ExitStack, tc: tile.TileContext, io: dict):
    nc = tc.nc

    consts = ctx.enter_context(tc.tile_pool(name="consts", bufs=1))
    wpool = ctx.enter_context(tc.tile_pool(name="wpool", bufs=1))
    big = ctx.enter_context(tc.tile_pool(name="big", bufs=1))
    h1p = ctx.enter_context(tc.tile_pool(name="h1p", bufs=2))
    msqp = ctx.enter_context(tc.tile_pool(name="msqp", bufs=2))
    expp = ctx.enter_context(tc.tile_pool(name="expp", bufs=1))
    v4p = ctx.enter_context(tc.tile_pool(name="v4p", bufs=2))
    epip = ctx.enter_context(tc.tile_pool(name="epip", bufs=2))
    gelp = ctx.enter_context(tc.tile_pool(name="gelp", bufs=2))
    outp = ctx.enter_context(tc.tile_pool(name="outp", bufs=2))

    # one PSUM pool, 4 tags totalling exactly 8 banks:
    #   qk01/qk23  each 1 x [128,1024] f32 (2 banks): head-pair QK scores,
    #              also LN stats pairs and MLP hidden halves
    #   A          2 x [128,512] f32 (2 banks): col-tiled AV accumulator
    #   lin        2 x [128,512] f32 (2 banks): linears, transposes
    psum = ctx.enter_context(tc.tile_pool(name="psum", bufs=1, space="PSUM"))

    def ps_pair(which, name):
        return psum.tile([128, 1024], F32, tag=f"qk{which}", bufs=1, name=name)

    def ps_lin(shape, name, dtype=F32):
        return psum.tile(shape, dtype, tag="lin", bufs=2, name=name)

    dma = nc.sync.dma_start
    mm = nc.tensor.matmul

    # ---------------- input DMAs (x first: LN1 starts on it) --------------
    xT = big.tile([128, L], F32, tag="xT")
    for t in range(NT):
        dma(xT[:, TT * t : TT * t + TT], io["x"][:, TT * t : TT * t + TT])

    # weight staging slab: 20 [128,128] blocks, transposed later on PE.
    # Wz/Wqkv first: the z-conditioned bias columns gate the qkv epilogues.
    wblocks = (
        [("Wz1", io["Wz1"][:, 128 * k : 128 * k + 128]) for k in range(4)]
        + [("Wz2", io["Wz2"][:, 128 * k : 128 * k + 128]) for k in range(4)]
        + [("Wqkv", io["Wqkv"][128 * j : 128 * j + 128, :]) for j in range(3)]
        + [("W1", io["W1"][128 * h : 128 * h + 128, :]) for h in range(4)]
        + [("W2", io["W2"][:, 128 * h : 128 * h + 128]) for h in range(4)]
        + [("Wproj", io["Wproj"][:, :])]
    )
    wst = wpool.tile([128, 128 * len(wblocks)], F32, tag="wst")
    for i, (_, src) in enumerate(wblocks):
        dma(wst[:, 128 * i : 128 * i + 128], src)

    def col(name):
        t = consts.tile([128, 1], F32, tag="col_" + name)
        dma(t[:], io[name].unsqueeze(1))
        return t

    g1c = col("ln1_g")
    bln1 = col("ln1_b")
    g2c = col("ln2_g")
    bln2 = col("ln2_b")
    bprojc = col("bproj")
    b2c = col("b2")

    b1cols = consts.tile([128, 4], F32, tag="b1cols")
    for h in range(4):
        dma(b1cols[:, h : h + 1], io["b1"][128 * h : 128 * h + 128].unsqueeze(1))
    zcols = consts.tile([128, 4], F32, tag="zcols")
    for k in range(4):
        dma(zcols[:, k : k + 1], io["z"][128 * k : 128 * k + 128].unsqueeze(1))

    # lepe bias as a column (rows 0:64 branch0, 64:128 branch1)
    lbias = consts.tile([128, 1], F32, tag="lbias")
    dma(lbias[0:CB, :], io["lepe_b0"].unsqueeze(1))
    dma(lbias[CB:128, :], io["lepe_b1"].unsqueeze(1))

    # conv tap weights: rows 0:64 branch-0 taps (a,b); rows 64:128 branch-1
    # taps transposed (branch-1 window images are stored transposed)
    wcomb = consts.tile([128, 9], F32, tag="wcomb")
    dma(wcomb[0:CB, :], io["lepe_w0"].rearrange("c o a b -> c (o a b)"))
    wtap = consts.tile([64, 9], F32, tag="wtap")
    dma(wtap[:], io["lepe_w1"].rearrange("c o a b -> c (o a b)"))

    # ---------------- constants ----------------
    ident = consts.tile([128, 128], F32, tag="ident")
    make_identity(nc, ident[:])
    identB = consts.tile([128, 128], BF16, tag="identB")
    nc.vector.tensor_copy(identB[:], ident[:])
    # 64x64 identity living at partitions 64:128 (for branch-1 v transposes)
    ident2f = consts.tile([128, 64], F32, tag="ident2f")
    nc.gpsimd.memset(ident2f[:], 0.0)
    nc.gpsimd.affine_select(
        out=ident2f[:], in_=ident2f[:], compare_op=OP.not_equal, fill=1.0,
        base=-64, pattern=[[-1, 64]], channel_multiplier=1,
    )
    ident2 = consts.tile([128, 64], BF16, tag="ident2")
    nc.vector.tensor_copy(ident2[:], ident2f[:])

    onesC = consts.tile([128, 128], F32, tag="onesC")
    nc.gpsimd.memset(onesC[:], 1.0 / C)
    # genuinely-rounded f32r copy: walrus requires f32r matmul operands to be
    # produced by an engine write with f32r output (bitcasts are rejected)
    onesCR = consts.tile([128, 128], F32R, tag="onesCR")
    nc.vector.tensor_copy(onesCR[:], onesC[:])
    epscol = consts.tile([128, 1], F32, tag="epscol")
    nc.gpsimd.memset(epscol[:], EPS)

    nc.gpsimd.tensor_copy(
        wcomb[CB:128, :].rearrange("c (b a) -> c b a", b=3),
        wtap[:].rearrange("c (a b) -> c a b", a=3).transpose([0, 2, 1]),
    )

    # v4 template: 32-wide head slots, col 16 ones (denominator), rest zero
    v4f = consts.tile([128, TT], F32, tag="v4f")
    nc.gpsimd.memset(v4f[:], 0.0)
    nc.vector.memset(
        v4f[:].rearrange("p (s w) -> p s w", s=16)[:, :, 16:17], 1.0
    )
    v4br = []
    for brr in range(2):
        v4 = consts.tile([128, TT], BF16, tag=f"v4br{brr}")
        nc.vector.tensor_copy(v4[:], v4f[:])
        v4br.append(v4)

    # ---------------- big activation buffers ----------------
    # head-strip padded q/k (per branch): head h at partitions 32h..32h+16
    qs = [
        big.tile([128, L], BF16, tag=f"qs{br}", name=f"qs{br}")
        for br in range(2)
    ]
    ks = [
        big.tile([128, L], BF16, tag=f"ks{br}", name=f"ks{br}")
        for br in range(2)
    ]
    vT = big.tile([128, L], BF16, tag="vT")
    cat = big.tile([128, L], BF16, tag="cat")
    xf2 = big.tile([128, L], F32R, tag="xf2")
    xc4 = big.tile([128, L], F32, tag="xc4")     # x - mean
    var4 = big.tile([128, L], F32, tag="var4")   # variance, then 1/sd
    sd4 = big.tile([128, L // 2], F32, tag="sd4")  # per-half sd scratch

    # lepe padded q image: [128, 66 outer, 8 windows x 10 inner]; interior of
    # window wj is [1+o, 10*wj + 1 + i]; zero columns isolate the windows.
    # rows 0:64 = branch-0 (h,w) raster; rows 64:128 = branch-1 transposed.
    PADI = 10
    PADO = 66
    pad = big.tile([128, PADO * NWIN * PADI], BF16, tag="pad")
    pad4 = pad[:].rearrange("c (o wj i) -> c o wj i", o=PADO, i=PADI)
    nc.gpsimd.memset(pad[:], 0.0)
    lepe = big.tile([128, L], BF16, tag="lepe")
    # branch-0 rows: index o*64 + wj*8 + i == h*64 + w  (L raster)
    # branch-1 rows: index o*64 + wj*8 + i == w*64 + h  (transposed raster)
    lepe4 = lepe[:].rearrange("c (o wj i) -> c o wj i", o=H, i=SS)

    # ---------------- layernorm: per-tile stats into shared buffers -------
    def ln_stats_tile(src, t):
        # The DMA'd f32 xT cannot feed an f32r matmul (walrus requires a
        # rounding engine write), so its sum goes through an ACT f32r copy.
        sl = slice(TT * t, TT * t + TT)
        xt = src[:, sl]
        xsq = msqp.tile([128, TT], F32R, tag="xsq")
        nc.gpsimd.tensor_mul(xsq[:], xt, xt)
        if xt.dtype == F32:
            xr = msqp.tile([128, TT], F32R, tag="lnscratch", name="xr")
            nc.scalar.copy(xr[:], xt)
            xmm = xr[:]
        else:
            xmm = xt
        s12 = ps_pair("01" if t % 2 == 0 else "23", "s12")
        mps = s12[:, 0:TT]
        s2ps = s12[:, TT : 2 * TT]
        mm(mps, onesCR[:], xmm, start=True, stop=True)
        mm(s2ps, onesCR[:], xsq[:], start=True, stop=True)
        msq = msqp.tile([128, TT], F32, tag="lnscratch", name="msq")
        nc.scalar.activation(msq[:], mps, AF.Square)
        nc.vector.tensor_sub(var4[:, sl], s2ps, msq[:])
        nc.vector.tensor_sub(xc4[:, sl], xt, mps)

    def ln_finish_half(hf):
        # per-half so downstream tiles 0-3 start while 4-7 stats still run;
        # Square sits in every ACT table so this adds no extra table loads
        sl = slice(2048 * hf, 2048 * hf + 2048)
        nc.scalar.activation(sd4[:, :], var4[:, sl], AF.Sqrt, bias=epscol[:])
        nc.vector.reciprocal_approx_fast(var4[:, sl], sd4[:, :])  # -> 1/sd

    # ---------------- phase 1: LN1 stats (overlaps x DMA) -----------------
    for t in range(NT):
        ln_stats_tile(xT, t)

    # ---------------- weight transposes (PE busy during LN tail) ---------
    WgT = wpool.tile([128, 3 * C], F32R, tag="WgT")
    WqkvT = wpool.tile([128, 3 * C], F32, tag="WqkvT")
    W1gT = wpool.tile([128, HID], F32R, tag="W1gT")
    W1T = wpool.tile([128, HID], F32, tag="W1T")
    W2T = wpool.tile([128, HID], BF16, tag="W2T")
    WpT = wpool.tile([128, C], BF16, tag="WpT")
    Wz1T = wpool.tile([128, CHZ], F32, tag="Wz1T")
    Wz2T = wpool.tile([128, CHZ], F32, tag="Wz2T")

    def wtrans(i, dst, scale_col=None, copy_dst=None):
        pt = ps_lin([128, 512], "ptw")[:, 0:128]
        nc.tensor.transpose(pt, wst[:, 128 * i : 128 * i + 128], ident[:])
        if scale_col is not None:
            nc.vector.tensor_scalar_mul(dst, pt, scale_col[:])
        else:
            nc.vector.tensor_copy(dst, pt)
        if copy_dst is not None:
            nc.vector.tensor_copy(copy_dst, pt)

    # ---------------- z-conditioned bias columns ----------------
    def zbias(WzT, blnc, tag):
        zp = ps_lin([128, 512], "zp")[:, 0:1]
        for k in range(4):
            mm(
                zp, WzT[:, 128 * k : 128 * k + 128], zcols[:, k : k + 1],
                start=(k == 0), stop=(k == 3),
            )
        bz = consts.tile([128, 1], F32, tag=tag)
        nc.vector.tensor_add(bz[:], zp, blnc[:])
        return bz

    for k in range(4):
        wtrans(k, Wz1T[:, 128 * k : 128 * k + 128])
        wtrans(4 + k, Wz2T[:, 128 * k : 128 * k + 128])
    bz1 = zbias(Wz1T, bln1, "bz1")
    bz2 = zbias(Wz2T, bln2, "bz2")
    for j in range(3):
        wtrans(8 + j, WgT[:, 128 * j : 128 * j + 128], g1c,
               WqkvT[:, 128 * j : 128 * j + 128])
    w0cols = consts.tile([128, 3], F32, tag="w0cols")
    for j in range(3):
        wp = ps_lin([128, 512], "wp")[:, 0:1]
        mm(wp, WqkvT[:, 128 * j : 128 * j + 128], bz1[:], start=True, stop=True)
        nc.vector.tensor_copy(w0cols[:, j : j + 1], wp)

    # head-strip stationaries for q/k: slot (32h+i), i<16 <- branch channel
    # 16h+i of Wqkv's q (or k) block; slots 16..31 of each strip stay zero so
    # the strip matmuls produce exact zeros in the pad partitions. The bias
    # path must use the UNSCALED weights (LN gain applies only to the
    # normalized input), so assemble unscaled first, then fold the gain.
    strip_w = []
    strip_b = []
    for qk in range(2):  # 0 = q, 1 = k
        for br in range(2):
            wsU = wpool.tile([128, 128], F32, tag=f"wstripU{qk}{br}")
            nc.vector.memset(wsU[:], 0.0)
            nc.vector.tensor_copy(
                wsU[:].rearrange("p (h s) -> p h s", h=4)[:, :, 0:D],
                WqkvT[:, 128 * qk + CB * br : 128 * qk + CB * br + CB].rearrange(
                    "p (h d) -> p h d", h=4
                ),
            )
            bp = ps_lin([128, 512], "bp")[:, 0:1]
            mm(bp, wsU[:], bz1[:], start=True, stop=True)
            bcol = consts.tile([128, 1], F32, tag=f"bstrip{qk}{br}")
            nc.vector.tensor_copy(bcol[:], bp)
            strip_b.append(bcol)
            wsR = wpool.tile([128, 128], F32R, tag=f"wstrip{qk}{br}")
            nc.vector.tensor_scalar_mul(wsR[:], wsU[:], g1c[:])
            strip_w.append(wsR)

    ln_finish_half(0)  # LN1 tiles 0-3 usable; PE keeps transposing below

    for h in range(4):
        wtrans(11 + h, W1gT[:, 128 * h : 128 * h + 128], g2c,
               W1T[:, 128 * h : 128 * h + 128])
    gbcols = consts.tile([128, 4], F32, tag="gbcols")
    for h in range(4):
        wp = ps_lin([128, 512], "wp")[:, 0:1]
        mm(wp, W1T[:, 128 * h : 128 * h + 128], bz2[:], start=True, stop=True)
        nc.vector.tensor_add(gbcols[:, h : h + 1], wp, b1cols[:, h : h + 1])
    for h in range(4):
        wtrans(15 + h, W2T[:, 128 * h : 128 * h + 128])
    wtrans(19, WpT[:])

    ln_finish_half(1)

    # ---------------- phase 2: qkv projections ----------------
    # per tile: q channel-major -> lepe pad interior; q/k head-strips; v.
    # token tile t covers image rows h in [8t, 8t+8).
    for t in range(NT):
        sl = slice(TT * t, TT * t + TT)
        hn = h1p.tile([128, TT], F32R, tag="hn")
        nc.vector.tensor_mul(hn[:], xc4[:, sl], var4[:, sl])

        # q channel-major, written straight into the padded lepe image
        qp = ps_lin([128, 512], "qp")
        mm(qp[:], WgT[:, 0:128], hn[:], start=True, stop=True)
        # branch 0 rows: (h, w) raster -> pad[0:64, 1+8t+hh, wj, 1+ww]
        nc.vector.tensor_scalar_add(
            pad4[0:CB, 8 * t + 1 : 8 * t + 9, :, 1 : 1 + SS],
            qp[0:CB, :].rearrange("c (hh wj ww) -> c hh wj ww", hh=SS, ww=SS),
            w0cols[0:CB, 0:1],
        )
        # branch 1 rows: transposed raster; tile t is exactly window wj=t
        nc.vector.tensor_scalar_add(
            pad4[CB:128, 1 : 1 + H, t, 1 : 1 + SS],
            qp[CB:128, :].rearrange("c (hh ww) -> c ww hh", hh=SS),
            w0cols[CB:128, 0:1],
        )

        # head-strip q/k for both branches. Branch-0 tensors are stored
        # WINDOW-MAJOR (window j's 512 tokens contiguous at cols 512j, in
        # (h, w%8) raster) so attention matmul operands are single-free-dim
        # APs (walrus rejects multi-dim moving/stationary APs there);
        # branch-1's L-raster is already window-contiguous.
        def win_major_dst(tens, psz=128):
            return tens[0:psz, :].rearrange(
                "p (wj hh ww) -> p wj hh ww", wj=NWIN, ww=SS
            )[:, :, SS * t : SS * t + SS, :]

        def win_major_src(ps, psz=128):
            return ps[0:psz, :].rearrange(
                "p (hh wj ww) -> p wj hh ww", hh=SS, ww=SS
            )

        for br in range(2):
            sp_ = ps_lin([128, 512], "sp_")
            mm(sp_[:], strip_w[br][:], hn[:], start=True, stop=True)
            sk_ = ps_lin([128, 512], "sk_")
            mm(sk_[:], strip_w[2 + br][:], hn[:], start=True, stop=True)
            if br == 0:
                nc.vector.tensor_scalar_add(
                    win_major_dst(qs[0]), win_major_src(sp_), strip_b[0][:]
                )
                nc.vector.tensor_scalar_add(
                    win_major_dst(ks[0]), win_major_src(sk_), strip_b[2][:]
                )
            else:
                nc.vector.tensor_scalar_add(qs[1][:, sl], sp_[:], strip_b[1][:])
                nc.vector.tensor_scalar_add(ks[1][:, sl], sk_[:], strip_b[3][:])

        vp = ps_lin([128, 512], "vp")
        mm(vp[:], WgT[:, 256:384], hn[:], start=True, stop=True)
        nc.vector.tensor_scalar_add(
            win_major_dst(vT, CB), win_major_src(vp, CB), w0cols[0:CB, 2:3]
        )
        nc.vector.tensor_scalar_add(
            vT[CB:128, sl], vp[CB:128, :], w0cols[CB:128, 2:3]
        )

    # ---------------- phase 2.5: LePE depthwise 3x3 over the padded image -
    # per-channel tap weight as a scalar column, shifted window reads,
    # in-place accumulate; zero separator columns isolate the windows.
    taps = [(a, b) for a in (-1, 0, 1) for b in (-1, 0, 1)]
    for idx, (a, b) in enumerate(taps):
        src = pad4[:, 1 + a : 1 + H + a, :, 1 + b : 1 + SS + b]
        wcol = wcomb[:, 3 * (a + 1) + (b + 1) : 3 * (a + 1) + (b + 2)]
        if idx == 0:
            nc.vector.tensor_scalar_mul(lepe4[:, :, :, :], src, wcol)
        else:
            nc.vector.scalar_tensor_tensor(
                lepe4[:, :, :, :], src, wcol, lepe4[:, :, :, :],
                op0=OP.mult, op1=OP.add,
            )

    # window access patterns -------------------------------------------------
    # branch 0: vertical strip window j = cols [8j,8j+8); raster (h, w)
    # branch 1: horizontal strip window j, stored transposed; raster (w, h)
    def win_ap(src, br, j, p0, psz):
        a3 = src[p0 : p0 + psz, :].rearrange("c (h w) -> c h w", h=H)
        if br == 0:
            return a3[:, :, SS * j : SS * j + SS]
        return a3[:, SS * j : SS * j + SS, :].transpose([0, 2, 1])

    # strip-layout window APs: both branches store window j's 512 tokens
    # contiguously at cols [512j, 512j+512)
    def strip_win(src, h, j):
        return src[32 * h : 32 * h + 32, TT * j : TT * j + TT]

    def strip_chunk(src, h, j, c):
        return src[32 * h : 32 * h + 32, TT * j + 128 * c : TT * j + 128 * c + 128]

    # ---------------- phase 3: attention, one (window, branch) at a time --
    for wb in range(2 * NWIN):
        j, br = wb // 2, wb % 2
        p0 = CB * br
        # v': token-major v (via PE transpose), into 32-wide head slots
        vps = ps_lin([128, 256], "vps", dtype=BF16)
        idv = identB[0:CB, 0:CB] if br == 0 else ident2[CB:128, :]
        for c in range(4):
            vsrc = vT[p0 : p0 + CB, TT * j + 128 * c : TT * j + 128 * c + 128]
            mm(
                vps[:, 64 * c : 64 * c + 64],
                vsrc,
                idv,
                is_transpose=True,
                start=(c == 0), stop=(c == 3),
            )
        v4 = v4br[br]
        nc.vector.tensor_copy(
            v4[:].rearrange("p (c h s) -> p c h s", c=4, h=4)[:, :, :, 0:D],
            vps[:].rearrange("p (c h d) -> p c h d", c=4, h=4),
        )

        # A: col-tiled AV accumulator, head h in partitions 32h..32h+32
        A = psum.tile([128, TT], F32, tag="A", bufs=2, name="A")
        for c in range(4):
            sc01 = ps_pair("01", "sc01")
            sc23 = ps_pair("23", "sc23")
            banks = {0: sc01[:, 0:512], 1: sc01[:, 512:1024],
                     2: sc23[:, 0:512], 3: sc23[:, 512:1024]}
            for h in range(NH):
                mm(
                    banks[h],
                    strip_chunk(ks[br], h, j, c),
                    strip_win(qs[br], h, j),
                    start=True, stop=True,
                    tile_position=(32 * h, 0),
                )
            es01 = expp.tile([128, 1024], BF16, tag="es01", bufs=2, name="es01")
            es23 = expp.tile([128, 1024], BF16, tag="es23", bufs=2, name="es23")
            nc.scalar.activation(es01[:], sc01[:], AF.Exp, scale=float(D) ** -0.5)
            nc.scalar.activation(es23[:], sc23[:], AF.Exp, scale=float(D) ** -0.5)
            espair = {0: es01, 1: es01, 2: es23, 3: es23}
            for h in range(NH):
                mm(
                    A[32 * h : 32 * h + 32, :],
                    v4[:, 128 * c + 32 * h : 128 * c + 32 * h + 32],
                    espair[h][:, 512 * (h % 2) : 512 * (h % 2) + 512],
                    start=(c == 0), stop=(c == 3),
                    tile_position=(0, 32 * h),
                    skip_group_check=True,
                )

        # epilogue: transpose -> divide by denominators -> transpose back
        Asb = epip.tile([128, TT], BF16, tag="Asb")
        nc.vector.tensor_copy(Asb[:], A[:])
        Tb = ps_lin([128, TT], "Tb", dtype=BF16)
        for c in range(4):
            mm(
                Tb[:, 128 * c : 128 * c + 128],
                Asb[:, 128 * c : 128 * c + 128],
                identB[:],
                is_transpose=True,
                start=(c == 0), stop=(c == 3),
            )
        Tv = Tb.rearrange("p (c h s) -> p c h s", c=4, h=4)
        R = epip.tile([128, 16], F32, tag="R")
        Rv = R[:].rearrange("p (c h) -> p c h", c=4)
        nc.vector.reciprocal(Rv[:, :, :], Tv[:, :, :, 16])
        E = epip.tile([128, 256], BF16, tag="E")
        Ev = E[:].rearrange("p (c h d) -> p c h d", c=4, h=4)
        nc.vector.tensor_mul(
            Ev[:, :, :, :],
            Tv[:, :, :, 0:D],
            Rv[:, :, :].unsqueeze(3).broadcast_to((128, 4, 4, D)),
        )
        Ot = ps_lin([CB, TT], "Ot", dtype=BF16)
        for c in range(4):
            mm(
                Ot[:, 128 * c : 128 * c + 128],
                E[:, 64 * c : 64 * c + 64],
                identB[:],
                is_transpose=True,
                start=(c == 0), stop=(c == 3),
            )
        # free the lin-tag PSUM slot promptly: the un-window below may wait
        # on the (long) LePE conv, and Ot parking in PSUM would stall the
        # attention pipeline via the lin tag.
        # staged at the branch's own partitions: the un-window STT needs all
        # its SBUF operands (Osb, lepe, cat) to share a start partition.
        Osb128 = epip.tile([128, TT], BF16, tag="Osb")
        Osb = Osb128[p0 : p0 + CB, :]
        nc.vector.tensor_copy(Osb, Ot[:])
        # un-window: (attention + lepe_bias) + lepe into cat rows.
        # branch 0 window tokens are (h, w)-rastered; branch 1 tokens are
        # L-contiguous (h, w) rows, with lepe stored (w, wj, h%8).
        cat3 = cat[p0 : p0 + CB, :].rearrange("c (h w) -> c h w", h=H)
        if br == 0:
            dst = cat3[:, :, SS * j : SS * j + SS]
            osrc = Osb.rearrange("c (h w) -> c h w", h=H)
            lsrc = lepe4[0:CB, :, j, :]
        else:
            dst = cat3[:, SS * j : SS * j + SS, :]
            osrc = Osb.rearrange("c (h w) -> c h w", h=SS)
            lsrc = lepe4[CB:128, :, j, :].transpose([0, 2, 1])
        nc.vector.scalar_tensor_tensor(
            dst, osrc, lbias[p0 : p0 + CB, :], lsrc,
            op0=OP.add, op1=OP.add,
        )

    # ---------------- phase 4: proj + residual + LN2 stats ----------------
    for t in range(NT):
        sl = slice(TT * t, TT * t + TT)
        ap_ = ps_lin([128, 512], "ap_")
        mm(ap_[:], WpT[:], cat[:, sl], start=True, stop=True)
        nc.vector.scalar_tensor_tensor(
            xf2[:, sl], ap_[:], bprojc[:], xT[:, sl], op0=OP.add, op1=OP.add
        )
        ln_stats_tile(xf2, t)
        if t == 3:
            ln_finish_half(0)  # MLP tiles 0-3 unblocked before proj 4-7 done
    ln_finish_half(1)

    # ---------------- phase 5: MLP + residual ----------------
    for t in range(NT):
        sl = slice(TT * t, TT * t + TT)
        hn = h1p.tile([128, TT], F32R, tag="hn")
        nc.vector.tensor_mul(hn[:], xc4[:, sl], var4[:, sl])
        o2 = ps_lin([128, 512], "o2")
        for half in range(2):
            hp = ps_pair("01" if half == 0 else "23", "hp")
            gel = gelp.tile([128, 2 * TT], BF16, tag="gel", bufs=2)
            for hh2 in range(2):
                hh = 2 * half + hh2
                mm(
                    hp[:, TT * hh2 : TT * hh2 + TT],
                    W1gT[:, 128 * hh : 128 * hh + 128],
                    hn[:],
                    start=True, stop=True,
                )
                nc.scalar.activation(
                    gel[:, TT * hh2 : TT * hh2 + TT],
                    hp[:, TT * hh2 : TT * hh2 + TT],
                    AF.Gelu,
                    bias=gbcols[:, hh : hh + 1],
                )
            for hh2 in range(2):
                hh = 2 * half + hh2
                mm(
                    o2[:],
                    W2T[:, 128 * hh : 128 * hh + 128],
                    gel[:, TT * hh2 : TT * hh2 + TT],
                    start=(hh == 0), stop=(hh == 3),
                )
        ot = outp.tile([128, TT], F32, tag="ot")
        nc.vector.scalar_tensor_tensor(
            ot[:], o2[:], b2c[:], xf2[:, sl], op0=OP.add, op1=OP.add
        )
        dma(io["out"][:, sl], ot[:])


_NC_CACHE = {}


def build_nc():
    key = "nc"
    if key in _NC_CACHE:
        return _NC_CACHE[key]
    nc = bacc.Bacc("TRN2", target_bir_lowering=False, debug=False)
    io = {}
    for name in INPUT_NAMES:
        io[name] = nc.dram_tensor(
            name, INPUT_SHAPES[name], F32, kind="ExternalInput"
        ).ap()
    io["out"] = nc.dram_tensor("out", [C, L], F32, kind="ExternalOutput").ap()
    with tile.TileContext(nc) as tc:
        with ExitStack() as ctx:
            emit(ctx, tc, io)
    nc.compile()
    _NC_CACHE[key] = nc
    return nc


def make_in_maps(inputs):
    in_maps = []
    for b in range(B):
        m = {
            "x": np.ascontiguousarray(
                inputs["x"][b].reshape(C, L).astype(np.float32)
            ),
            "z": np.ascontiguousarray(inputs["z"][b].astype(np.float32)),
        }
        for name in INPUT_NAMES:
            if name in ("x", "z"):
                continue
            m[name] = np.ascontiguousarray(np.asarray(inputs[name], np.float32))
        in_maps.append(m)
    return in_maps


def kernel(**inputs):
    nc = build_nc()
    in_maps = make_in_maps(inputs)
    res = bass_utils.run_bass_kernel_spmd(nc, in_maps, list(range(B)))
    out = np.stack([res.results[b]["out"].reshape(C, H, W) for b in range(B)])
    return out.astype(np.float32)


if __name__ == "__main__":
    # CoreSim numerics check of core 0 against the reference (dev only).
    import sys

    sys.path.insert(0, "/root/problem")
    import reference

    from concourse.bass_interp import CoreSim

    # CoreSim has no Gelu; patch it (HW has a native erf-gelu table).
    import scipy.special
    from concourse import bass_interp

    _orig_act = bass_interp.InstructionExecutor.visit_InstActivation

    def _patched_act(self, instruction, *, reg_snapshot=None):
        if instruction.func == mybir.ActivationFunctionType.Gelu:
            instruction.func = mybir.ActivationFunctionType.Identity
            try:
                _orig_act(self, instruction, reg_snapshot=reg_snapshot)
            finally:
                instruction.func = mybir.ActivationFunctionType.Gelu
            ov = self.view_ap(
                instruction.outs[0],
                bass_interp.Direction.WRITE,
                instruction,
                reg_snapshot=reg_snapshot,
            )
            x = ov.astype(np.float64)
            ov[:] = (
                x * 0.5 * (1.0 + scipy.special.erf(x / np.sqrt(2.0)))
            ).astype(np.float32)
            return
        return _orig_act(self, instruction, reg_snapshot=reg_snapshot)

    bass_interp.InstructionExecutor.visit_InstActivation = _patched_act

    inputs = {k: np.asarray(v) for k, v in reference.setup_inputs().items()}
    expected = np.asarray(reference.reference(**inputs))

    nc = build_nc()
    print("built+compiled", flush=True)
    sim = CoreSim(nc, require_finite=True, require_nnan=True)
    m = make_in_maps(inputs)[0]
    for k, v in m.items():
        sim.tensor(k)[:] = v
    sim.simulate(check_with_hw=False)
    got = sim.tensor("out").reshape(C, H, W)
    exp0 = expected[0]
    err = np.abs(got - exp0)
    denom = np.abs(exp0).max()
    print("absmax err:", err.max(), "rel:", err.max() / denom)
    print(
        "rms rel:",
        np.sqrt(((got - exp0) ** 2).mean()) / np.sqrt((exp0**2).mean()),
    )


# revision 21
# speedup vs baseline: 1.2609x; 1.0193x over previous
"""Trainium2 Bass kernel for DegradationRectifyNet block (CSWin-style window
attention + LePE depthwise conv + code-conditioned LN/MLP).

Data-parallel over batch B=8 across 8 NeuronCores; one image per core.
On-chip everything is channel-major (C on partitions, tokens on free dim).

v3 restructure (vs v2 461us baseline): the PE was HAM-throttled to 1.2GHz
for 80% of the run and streamed 262k of its 342k rows in per-head QK/AV
matmuls that used only 16 of 128 contraction rows (QK) or 32 of 128
stationary columns (AV).
  - q/k are projected directly into per-branch "head-strip" layouts
    (head h lives at partitions 32h..32h+16, rest zero), so the four
    heads' QK matmuls run CONCURRENTLY as 32-row tile_position row tiles
    (4x fewer PE cycles), reading window tokens via strided APs (the old
    SBUF->SBUF window-gather DMAs disappear).
  - AV is col-tiled: each head's v-stationary writes its own 32-partition
    strip of one accumulating PSUM bank (4x fewer PE cycles), which also
    kills the per-head avh->A copies.
  - Scores live in two [128,1024] head-pair PSUM tags so ACT exp runs as
    [128,1024] instructions while the next chunk's QK fills the other
    pair; ACT-exp is the new critical resource (~16.8M elements).
  - LePE runs once over a zero-separated padded image ([128, 66, 8x10])
    written in-place by the qkv epilogue, so windows need no per-window
    pad/memset and window isolation comes from the zero columns.
"""

import numpy as np

import concourse.bass as bass
import concourse.bacc as bacc
import concourse.tile as tile
from concourse import mybir
from concourse import bass_utils
from concourse.masks import make_identity
from contextlib import ExitStack

F32 = mybir.dt.float32
F32R = mybir.dt.float32r
BF16 = mybir.dt.bfloat16
AF = mybir.ActivationFunctionType
OP = mybir.AluOpType

B = 8
C = 128          # channels
H = W = 64
L = H * W        # 4096 tokens
SS = 8           # split size
CB = C // 2      # branch channels = 64
NH = 4           # heads per branch
D = CB // NH     # head dim = 16
HID = 4 * C      # 512
CHZ = 512
NT = 8           # token tiles
TT = 512         # tokens per tile
NWIN = 8         # windows per branch
EPS = 1e-5

INPUT_NAMES = [
    "x", "z", "ln1_g", "ln1_b", "ln2_g", "ln2_b", "Wz1", "Wz2", "Wqkv",
    "Wproj", "bproj", "lepe_w0", "lepe_b0", "lepe_w1", "lepe_b1",
    "W1", "b1", "W2", "b2",
]
INPUT_SHAPES = {
    "x": [C, L], "z": [CHZ],
    "ln1_g": [C], "ln1_b": [C], "ln2_g": [C], "ln2_b": [C],
    "Wz1": [C, CHZ], "Wz2": [C, CHZ], "Wqkv": [3 * C, C],
    "Wproj": [C, C], "bproj": [C],
    "lepe_w0": [CB, 1, 3, 3], "lepe_b0": [CB],
    "lepe_w1": [CB, 1, 3, 3], "lepe_b1": [CB],
    "W1": [HID, C], "b1": [HID], "W2": [C, HID], "b2": [C],
}


def emit(ctx: ExitStack, tc: tile.TileContext, io: dict):
    nc = tc.nc

    consts = ctx.enter_context(tc.tile_pool(name="consts", bufs=1))
    wpool = ctx.enter_context(tc.tile_pool(name="wpool", bufs=1))
    big = ctx.enter_context(tc.tile_pool(name="big", bufs=1))
    h1p = ctx.enter_context(tc.tile_pool(name="h1p", bufs=2))
    msqp = ctx.enter_context(tc.tile_pool(name="msqp", bufs=2))
    expp = ctx.enter_context(tc.tile_pool(name="expp", bufs=1))
    v4p = ctx.enter_context(tc.tile_pool(name="v4p", bufs=2))
    epip = ctx.enter_context(tc.tile_pool(name="epip", bufs=2))
    gelp = ctx.enter_context(tc.tile_pool(name="gelp", bufs=2))
    outp = ctx.enter_context(tc.tile_pool(name="outp", bufs=2))

    # one PSUM pool, 4 tags totalling exactly 8 banks:
    #   qk01/qk23  each 1 x [128,1024] f32 (2 banks): head-pair QK scores,
    #              also LN stats pairs and MLP hidden halves
    #   A          2 x [128,512] f32 (2 banks): col-tiled AV accumulator
    #   lin        2 x [128,512] f32 (2 banks): linears, transposes
    psum = ctx.enter_context(tc.tile_pool(name="psum", bufs=1, space="PSUM"))

    def ps_pair(which, name):
        return psum.tile([128, 1024], F32, tag=f"qk{which}", bufs=1, name=name)

    def ps_lin(shape, name, dtype=F32):
        return psum.tile(shape, dtype, tag="lin", bufs=2, name=name)

    dma = nc.sync.dma_start
    mm = nc.tensor.matmul

    # ---------------- input DMAs (x first: LN1 starts on it) --------------
    xT = big.tile([128, L], F32, tag="xT")
    for t in range(NT):
        dma(xT[:, TT * t : TT * t + TT], io["x"][:, TT * t : TT * t + TT])

    # weight staging slab: 20 [128,128] blocks, transposed later on PE.
    # Wz/Wqkv first: the z-conditioned bias columns gate the qkv epilogues.
    wblocks = (
        [("Wz1", io["Wz1"][:, 128 * k : 128 * k + 128]) for k in range(4)]
        + [("Wz2", io["Wz2"][:, 128 * k : 128 * k + 128]) for k in range(4)]
        + [("Wqkv", io["Wqkv"][128 * j : 128 * j + 128, :]) for j in range(3)]
        + [("W1", io["W1"][128 * h : 128 * h + 128, :]) for h in range(4)]
        + [("W2", io["W2"][:, 128 * h : 128 * h + 128]) for h in range(4)]
        + [("Wproj", io["Wproj"][:, :])]
    )
    wst = wpool.tile([128, 128 * len(wblocks)], F32, tag="wst")
    for i, (_, src) in enumerate(wblocks):
        dma(wst[:, 128 * i : 128 * i + 128], src)

    def col(name):
        t = consts.tile([128, 1], F32, tag="col_" + name)
        dma(t[:], io[name].unsqueeze(1))
        return t

    g1c = col("ln1_g")
    bln1 = col("ln1_b")
    g2c = col("ln2_g")
    bln2 = col("ln2_b")
    bprojc = col("bproj")
    b2c = col("b2")

    b1cols = consts.tile([128, 4], F32, tag="b1cols")
    for h in range(4):
        dma(b1cols[:, h : h + 1], io["b1"][128 * h : 128 * h + 128].unsqueeze(1))
    zcols = consts.tile([128, 4], F32, tag="zcols")
    for k in range(4):
        dma(zcols[:, k : k + 1], io["z"][128 * k : 128 * k + 128].unsqueeze(1))

    # lepe bias as a column (rows 0:64 branch0, 64:128 branch1)
    lbias = consts.tile([128, 1], F32, tag="lbias")
    dma(lbias[0:CB, :], io["lepe_b0"].unsqueeze(1))
    dma(lbias[CB:128, :], io["lepe_b1"].unsqueeze(1))

    # conv tap weights: rows 0:64 branch-0 taps (a,b); rows 64:128 branch-1
    # taps transposed (branch-1 window images are stored transposed)
    wcomb = consts.tile([128, 9], F32, tag="wcomb")
    dma(wcomb[0:CB, :], io["lepe_w0"].rearrange("c o a b -> c (o a b)"))
    wtap = consts.tile([64, 9], F32, tag="wtap")
    dma(wtap[:], io["lepe_w1"].rearrange("c o a b -> c (o a b)"))

    # ---------------- constants ----------------
    ident = consts.tile([128, 128], F32, tag="ident")
    make_identity(nc, ident[:])
    identB = consts.tile([128, 128], BF16, tag="identB")
    nc.vector.tensor_copy(identB[:], ident[:])
    # 64x64 identity living at partitions 64:128 (for branch-1 v transposes)
    ident2f = consts.tile([128, 64], F32, tag="ident2f")
    nc.gpsimd.memset(ident2f[:], 0.0)
    nc.gpsimd.affine_select(
        out=ident2f[:], in_=ident2f[:], compare_op=OP.not_equal, fill=1.0,
        base=-64, pattern=[[-1, 64]], channel_multiplier=1,
    )
    ident2 = consts.tile([128, 64], BF16, tag="ident2")
    nc.vector.tensor_copy(ident2[:], ident2f[:])

    onesC = consts.tile([128, 128], F32, tag="onesC")
    nc.gpsimd.memset(onesC[:], 1.0 / C)
    # genuinely-rounded f32r copy: walrus requires f32r matmul operands to be
    # produced by an engine write with f32r output (bitcasts are rejected)
    onesCR = consts.tile([128, 128], F32R, tag="onesCR")
    nc.vector.tensor_copy(onesCR[:], onesC[:])
    epscol = consts.tile([128, 1], F32, tag="epscol")
    nc.gpsimd.memset(epscol[:], EPS)

    nc.gpsimd.tensor_copy(
        wcomb[CB:128, :].rearrange("c (b a) -> c b a", b=3),
        wtap[:].rearrange("c (a b) -> c a b", a=3).transpose([0, 2, 1]),
    )

    # v4 template: 32-wide head slots, col 16 ones (denominator), rest zero
    v4f = consts.tile([128, TT], F32, tag="v4f")
    nc.gpsimd.memset(v4f[:], 0.0)
    nc.vector.memset(
        v4f[:].rearrange("p (s w) -> p s w", s=16)[:, :, 16:17], 1.0
    )
    v4br = []
    for brr in range(2):
        v4 = consts.tile([128, TT], BF16, tag=f"v4br{brr}")
        nc.vector.tensor_copy(v4[:], v4f[:])
        v4br.append(v4)

    # ---------------- big activation buffers ----------------
    # head-strip padded q/k (per branch): head h at partitions 32h..32h+16
    qs = [
        big.tile([128, L], BF16, tag=f"qs{br}", name=f"qs{br}")
        for br in range(2)
    ]
    ks = [
        big.tile([128, L], BF16, tag=f"ks{br}", name=f"ks{br}")
        for br in range(2)
    ]
    vT = big.tile([128, L], BF16, tag="vT")
    cat = big.tile([128, L], BF16, tag="cat")
    xf2 = big.tile([128, L], F32R, tag="xf2")
    xc4 = big.tile([128, L], F32, tag="xc4")     # x - mean
    var4 = big.tile([128, L], F32, tag="var4")   # variance, then 1/sd
    sd4 = big.tile([128, L // 2], F32, tag="sd4")  # per-half sd scratch

    # lepe padded q image: [128, 66 outer, 8 windows x 10 inner]; interior of
    # window wj is [1+o, 10*wj + 1 + i]; zero columns isolate the windows.
    # rows 0:64 = branch-0 (h,w) raster; rows 64:128 = branch-1 transposed.
    PADI = 10
    PADO = 66
    pad = big.tile([128, PADO * NWIN * PADI], BF16, tag="pad")
    pad4 = pad[:].rearrange("c (o wj i) -> c o wj i", o=PADO, i=PADI)
    nc.gpsimd.memset(pad[:], 0.0)
    lepe = big.tile([128, L], BF16, tag="lepe")
    # branch-0 rows: index o*64 + wj*8 + i == h*64 + w  (L raster)
    # branch-1 rows: index o*64 + wj*8 + i == w*64 + h  (transposed raster)
    lepe4 = lepe[:].rearrange("c (o wj i) -> c o wj i", o=H, i=SS)

    # ---------------- layernorm: per-tile stats into shared buffers -------
    def ln_stats_tile(src, t):
        # The DMA'd f32 xT cannot feed an f32r matmul (walrus requires a
        # rounding engine write), so its sum goes through an ACT f32r copy.
        sl = slice(TT * t, TT * t + TT)
        xt = src[:, sl]
        xsq = msqp.tile([128, TT], F32R, tag="xsq")
        nc.gpsimd.tensor_mul(xsq[:], xt, xt)
        if xt.dtype == F32:
            xr = msqp.tile([128, TT], F32R, tag="lnscratch", name="xr")
            nc.scalar.copy(xr[:], xt)
            xmm = xr[:]
        else:
            xmm = xt
        s12 = ps_pair("01" if t % 2 == 0 else "23", "s12")
        mps = s12[:, 0:TT]
        s2ps = s12[:, TT : 2 * TT]
        mm(mps, onesCR[:], xmm, start=True, stop=True)
        mm(s2ps, onesCR[:], xsq[:], start=True, stop=True)
        msq = msqp.tile([128, TT], F32, tag="lnscratch", name="msq")
        nc.scalar.activation(msq[:], mps, AF.Square)
        nc.vector.tensor_sub(var4[:, sl], s2ps, msq[:])
        nc.vector.tensor_sub(xc4[:, sl], xt, mps)

    def ln_finish_half(hf):
        # per-half so downstream tiles 0-3 start while 4-7 stats still run;
        # Square sits in every ACT table so this adds no extra table loads
        sl = slice(2048 * hf, 2048 * hf + 2048)
        nc.scalar.activation(sd4[:, :], var4[:, sl], AF.Sqrt, bias=epscol[:])
        nc.vector.reciprocal_approx_fast(var4[:, sl], sd4[:, :])  # -> 1/sd

    # ---------------- phase 1: LN1 stats (overlaps x DMA) -----------------
    for t in range(NT):
        ln_stats_tile(xT, t)

    # ---------------- weight transposes (PE busy during LN tail) ---------
    WgT = wpool.tile([128, 3 * C], F32R, tag="WgT")
    WqkvT = wpool.tile([128, 3 * C], F32, tag="WqkvT")
    W1gT = wpool.tile([128, HID], F32R, tag="W1gT")
    W1T = wpool.tile([128, HID], F32, tag="W1T")
    W2T = wpool.tile([128, HID], BF16, tag="W2T")
    WpT = wpool.tile([128, C], BF16, tag="WpT")
    Wz1T = wpool.tile([128, CHZ], F32, tag="Wz1T")
    Wz2T = wpool.tile([128, CHZ], F32, tag="Wz2T")

    def wtrans(i, dst, scale_col=None, copy_dst=None):
        pt = ps_lin([128, 512], "ptw")[:, 0:128]
        nc.tensor.transpose(pt, wst[:, 128 * i : 128 * i + 128], ident[:])
        if scale_col is not None:
            nc.vector.tensor_scalar_mul(dst, pt, scale_col[:])
        else:
            nc.vector.tensor_copy(dst, pt)
        if copy_dst is not None:
            nc.vector.tensor_copy(copy_dst, pt)

    # ---------------- z-conditioned bias columns ----------------
    def zbias(WzT, blnc, tag):
        zp = ps_lin([128, 512], "zp")[:, 0:1]
        for k in range(4):
            mm(
                zp, WzT[:, 128 * k : 128 * k + 128], zcols[:, k : k + 1],
                start=(k == 0), stop=(k == 3),
            )
        bz = consts.tile([128, 1], F32, tag=tag)
        nc.vector.tensor_add(bz[:], zp, blnc[:])
        return bz

    for k in range(4):
        wtrans(k, Wz1T[:, 128 * k : 128 * k + 128])
        wtrans(4 + k, Wz2T[:, 128 * k : 128 * k + 128])
    bz1 = zbias(Wz1T, bln1, "bz1")
    bz2 = zbias(Wz2T, bln2, "bz2")
    for j in range(3):
        wtrans(8 + j, WgT[:, 128 * j : 128 * j + 128], g1c,
               WqkvT[:, 128 * j : 128 * j + 128])
    w0cols = consts.tile([128, 3], F32, tag="w0cols")
    for j in range(3):
        wp = ps_lin([128, 512], "wp")[:, 0:1]
        mm(wp, WqkvT[:, 128 * j : 128 * j + 128], bz1[:], start=True, stop=True)
        nc.vector.tensor_copy(w0cols[:, j : j + 1], wp)

    # head-strip stationaries for q/k: slot (32h+i), i<16 <- branch channel
    # 16h+i of Wqkv's q (or k) block; slots 16..31 of each strip stay zero so
    # the strip matmuls produce exact zeros in the pad partitions. The bias
    # path must use the UNSCALED weights (LN gain applies only to the
    # normalized input), so assemble unscaled first, then fold the gain.
    strip_w = []
    strip_b = []
    for qk in range(2):  # 0 = q, 1 = k
        for br in range(2):
            wsU = wpool.tile([128, 128], F32, tag=f"wstripU{qk}{br}")
            nc.vector.memset(wsU[:], 0.0)
            nc.vector.tensor_copy(
                wsU[:].rearrange("p (h s) -> p h s", h=4)[:, :, 0:D],
                WqkvT[:, 128 * qk + CB * br : 128 * qk + CB * br + CB].rearrange(
                    "p (h d) -> p h d", h=4
                ),
            )
            bp = ps_lin([128, 512], "bp")[:, 0:1]
            mm(bp, wsU[:], bz1[:], start=True, stop=True)
            bcol = consts.tile([128, 1], F32, tag=f"bstrip{qk}{br}")
            nc.vector.tensor_copy(bcol[:], bp)
            strip_b.append(bcol)
            wsR = wpool.tile([128, 128], F32R, tag=f"wstrip{qk}{br}")
            nc.vector.tensor_scalar_mul(wsR[:], wsU[:], g1c[:])
            strip_w.append(wsR)

    ln_finish_half(0)  # LN1 tiles 0-3 usable; PE keeps transposing below

    for h in range(4):
        wtrans(11 + h, W1gT[:, 128 * h : 128 * h + 128], g2c,
               W1T[:, 128 * h : 128 * h + 128])
    gbcols = consts.tile([128, 4], F32, tag="gbcols")
    for h in range(4):
        wp = ps_lin([128, 512], "wp")[:, 0:1]
        mm(wp, W1T[:, 128 * h : 128 * h + 128], bz2[:], start=True, stop=True)
        nc.vector.tensor_add(gbcols[:, h : h + 1], wp, b1cols[:, h : h + 1])
    for h in range(4):
        wtrans(15 + h, W2T[:, 128 * h : 128 * h + 128])
    wtrans(19, WpT[:])

    ln_finish_half(1)

    # ---------------- phase 2: qkv projections ----------------
    # per tile: q channel-major -> lepe pad interior; q/k head-strips; v.
    # token tile t covers image rows h in [8t, 8t+8).
    for t in range(NT):
        sl = slice(TT * t, TT * t + TT)
        hn = h1p.tile([128, TT], F32R, tag="hn")
        nc.vector.tensor_mul(hn[:], xc4[:, sl], var4[:, sl])

        # q channel-major, written straight into the padded lepe image.
        # These (and the branch-0 v write below) run on ACT — Identity sits
        # in every table set and ACT is otherwise idle during this phase,
        # while DVE is the qkv-epilogue bottleneck.
        qp = ps_lin([128, 512], "qp")
        mm(qp[:], WgT[:, 0:128], hn[:], start=True, stop=True)
        # branch 0 rows: (h, w) raster -> pad[0:64, 1+8t+hh, wj, 1+ww]
        nc.scalar.activation(
            pad4[0:CB, 8 * t + 1 : 8 * t + 9, :, 1 : 1 + SS],
            qp[0:CB, :].rearrange("c (hh wj ww) -> c hh wj ww", hh=SS, ww=SS),
            AF.Identity,
            bias=w0cols[0:CB, 0:1],
        )
        # branch 1 rows: transposed raster; tile t is exactly window wj=t
        nc.scalar.activation(
            pad4[CB:128, 1 : 1 + H, t, 1 : 1 + SS],
            qp[CB:128, :].rearrange("c (hh ww) -> c ww hh", hh=SS),
            AF.Identity,
            bias=w0cols[CB:128, 0:1],
        )

        # head-strip q/k for both branches. Branch-0 tensors are stored
        # WINDOW-MAJOR (window j's 512 tokens contiguous at cols 512j, in
        # (h, w%8) raster) so attention matmul operands are single-free-dim
        # APs (walrus rejects multi-dim moving/stationary APs there);
        # branch-1's L-raster is already window-contiguous.
        def win_major_dst(tens, psz=128):
            return tens[0:psz, :].rearrange(
                "p (wj hh ww) -> p wj hh ww", wj=NWIN, ww=SS
            )[:, :, SS * t : SS * t + SS, :]

        def win_major_src(ps, psz=128):
            return ps[0:psz, :].rearrange(
                "p (hh wj ww) -> p wj hh ww", hh=SS, ww=SS
            )

        for br in range(2):
            sp_ = ps_lin([128, 512], "sp_")
            mm(sp_[:], strip_w[br][:], hn[:], start=True, stop=True)
            sk_ = ps_lin([128, 512], "sk_")
            mm(sk_[:], strip_w[2 + br][:], hn[:], start=True, stop=True)
            if br == 0:
                nc.vector.tensor_scalar_add(
                    win_major_dst(qs[0]), win_major_src(sp_), strip_b[0][:]
                )
                nc.vector.tensor_scalar_add(
                    win_major_dst(ks[0]), win_major_src(sk_), strip_b[2][:]
                )
            else:
                nc.vector.tensor_scalar_add(qs[1][:, sl], sp_[:], strip_b[1][:])
                nc.vector.tensor_scalar_add(ks[1][:, sl], sk_[:], strip_b[3][:])

        vp = ps_lin([128, 512], "vp")
        mm(vp[:], WgT[:, 256:384], hn[:], start=True, stop=True)
        nc.scalar.activation(
            win_major_dst(vT, CB), win_major_src(vp, CB), AF.Identity,
            bias=w0cols[0:CB, 2:3],
        )
        nc.vector.tensor_scalar_add(
            vT[CB:128, sl], vp[CB:128, :], w0cols[CB:128, 2:3]
        )

    # ---------------- phase 2.5: LePE depthwise 3x3 over the padded image -
    # per-channel tap weight as a scalar column, shifted window reads,
    # in-place accumulate; zero separator columns isolate the windows.
    taps = [(a, b) for a in (-1, 0, 1) for b in (-1, 0, 1)]
    for idx, (a, b) in enumerate(taps):
        src = pad4[:, 1 + a : 1 + H + a, :, 1 + b : 1 + SS + b]
        wcol = wcomb[:, 3 * (a + 1) + (b + 1) : 3 * (a + 1) + (b + 2)]
        if idx == 0:
            nc.vector.tensor_scalar_mul(lepe4[:, :, :, :], src, wcol)
        else:
            nc.vector.scalar_tensor_tensor(
                lepe4[:, :, :, :], src, wcol, lepe4[:, :, :, :],
                op0=OP.mult, op1=OP.add,
            )

    # window access patterns -------------------------------------------------
    # branch 0: vertical strip window j = cols [8j,8j+8); raster (h, w)
    # branch 1: horizontal strip window j, stored transposed; raster (w, h)
    def win_ap(src, br, j, p0, psz):
        a3 = src[p0 : p0 + psz, :].rearrange("c (h w) -> c h w", h=H)
        if br == 0:
            return a3[:, :, SS * j : SS * j + SS]
        return a3[:, SS * j : SS * j + SS, :].transpose([0, 2, 1])

    # strip-layout window APs: both branches store window j's 512 tokens
    # contiguously at cols [512j, 512j+512)
    def strip_win(src, h, j):
        return src[32 * h : 32 * h + 32, TT * j : TT * j + TT]

    def strip_chunk(src, h, j, c):
        return src[32 * h : 32 * h + 32, TT * j + 128 * c : TT * j + 128 * c + 128]

    # ---------------- phase 3: attention, one (window, branch) at a time --
    # The per-wb PE side work (v-transposes for the next wb, epilogue
    # transposes of the previous wb) is interleaved into the chunk loop so
    # it fills the PE between QK/AV bursts instead of parking at the wb
    # boundary and starving ACT-exp (the critical engine here).
    def emit_v4(wb):
        j, br = wb // 2, wb % 2
        p0 = CB * br
        vps = ps_lin([128, 256], "vps", dtype=BF16)
        idv = identB[0:CB, 0:CB] if br == 0 else ident2[CB:128, :]
        for c in range(4):
            vsrc = vT[p0 : p0 + CB, TT * j + 128 * c : TT * j + 128 * c + 128]
            mm(
                vps[:, 64 * c : 64 * c + 64],
                vsrc,
                idv,
                is_transpose=True,
                start=(c == 0), stop=(c == 3),
            )
        v4 = v4br[br]
        nc.vector.tensor_copy(
            v4[:].rearrange("p (c h s) -> p c h s", c=4, h=4)[:, :, :, 0:D],
            vps[:].rearrange("p (c h d) -> p c h d", c=4, h=4),
        )

    def epi_a(pend):
        # transpose A to token-major
        A, j, br = pend
        Asb = epip.tile([128, TT], BF16, tag="Asb")
        nc.vector.tensor_copy(Asb[:], A[:])
        Tb = ps_lin([128, TT], "Tb", dtype=BF16)
        for c in range(4):
            mm(
                Tb[:, 128 * c : 128 * c + 128],
                Asb[:, 128 * c : 128 * c + 128],
                identB[:],
                is_transpose=True,
                start=(c == 0), stop=(c == 3),
            )
        return Tb

    def epi_b(pend, Tb):
        # divide by denominators, transpose back, un-window + lepe
        A, j, br = pend
        p0 = CB * br
        Tv = Tb.rearrange("p (c h s) -> p c h s", c=4, h=4)
        R = epip.tile([128, 16], F32, tag="R")
        Rv = R[:].rearrange("p (c h) -> p c h", c=4)
        nc.vector.reciprocal(Rv[:, :, :], Tv[:, :, :, 16])
        E = epip.tile([128, 256], BF16, tag="E")
        Ev = E[:].rearrange("p (c h d) -> p c h d", c=4, h=4)
        nc.vector.tensor_mul(
            Ev[:, :, :, :],
            Tv[:, :, :, 0:D],
            Rv[:, :, :].unsqueeze(3).broadcast_to((128, 4, 4, D)),
        )
        Ot = ps_lin([CB, TT], "Ot", dtype=BF16)
        for c in range(4):
            mm(
                Ot[:, 128 * c : 128 * c + 128],
                E[:, 64 * c : 64 * c + 64],
                identB[:],
                is_transpose=True,
                start=(c == 0), stop=(c == 3),
            )
        # copy out of PSUM promptly (the un-window below may wait on the
        # LePE conv; Ot parking in PSUM would stall the lin tag), staged at
        # the branch's partitions (STT SBUF operands share a start partition)
        Osb128 = epip.tile([128, TT], BF16, tag="Osb")
        Osb = Osb128[p0 : p0 + CB, :]
        nc.vector.tensor_copy(Osb, Ot[:])
        cat3 = cat[p0 : p0 + CB, :].rearrange("c (h w) -> c h w", h=H)
        if br == 0:
            dst = cat3[:, :, SS * j : SS * j + SS]
            osrc = Osb.rearrange("c (h w) -> c h w", h=H)
            lsrc = lepe4[0:CB, :, j, :]
        else:
            dst = cat3[:, SS * j : SS * j + SS, :]
            osrc = Osb.rearrange("c (h w) -> c h w", h=SS)
            lsrc = lepe4[CB:128, :, j, :].transpose([0, 2, 1])
        nc.vector.scalar_tensor_tensor(
            dst, osrc, lbias[p0 : p0 + CB, :], lsrc,
            op0=OP.add, op1=OP.add,
        )

    emit_v4(0)
    pend = None
    Tb_pend = None
    for wb in range(2 * NWIN):
        j, br = wb // 2, wb % 2
        # A: col-tiled AV accumulator, head h in partitions 32h..32h+32
        A = psum.tile([128, TT], F32, tag="A", bufs=2, name="A")
        v4 = v4br[br]
        for c in range(4):
            sc01 = ps_pair("01", "sc01")
            sc23 = ps_pair("23", "sc23")
            banks = {0: sc01[:, 0:512], 1: sc01[:, 512:1024],
                     2: sc23[:, 0:512], 3: sc23[:, 512:1024]}
            for h in range(NH):
                mm(
                    banks[h],
                    strip_chunk(ks[br], h, j, c),
                    strip_win(qs[br], h, j),
                    start=True, stop=True,
                    tile_position=(32 * h, 0),
                )
            es01 = expp.tile([128, 1024], BF16, tag="es01", bufs=2, name="es01")
            es23 = expp.tile([128, 1024], BF16, tag="es23", bufs=2, name="es23")
            nc.scalar.activation(es01[:], sc01[:], AF.Exp, scale=float(D) ** -0.5)
            nc.scalar.activation(es23[:], sc23[:], AF.Exp, scale=float(D) ** -0.5)
            espair = {0: es01, 1: es01, 2: es23, 3: es23}
            for h in range(NH):
                mm(
                    A[32 * h : 32 * h + 32, :],
                    v4[:, 128 * c + 32 * h : 128 * c + 32 * h + 32],
                    espair[h][:, 512 * (h % 2) : 512 * (h % 2) + 512],
                    start=(c == 0), stop=(c == 3),
                    tile_position=(0, 32 * h),
                    skip_group_check=True,
                )
            if c == 0 and pend is not None:
                Tb_pend = epi_a(pend)
            elif c == 1 and pend is not None:
                epi_b(pend, Tb_pend)
            elif c == 2 and wb + 1 < 2 * NWIN:
                emit_v4(wb + 1)
        pend = (A, j, br)
    Tb_pend = epi_a(pend)
    epi_b(pend, Tb_pend)

    # ---------------- phase 4: proj + residual + LN2 stats ----------------
    for t in range(NT):
        sl = slice(TT * t, TT * t + TT)
        ap_ = ps_lin([128, 512], "ap_")
        mm(ap_[:], WpT[:], cat[:, sl], start=True, stop=True)
        nc.vector.scalar_tensor_tensor(
            xf2[:, sl], ap_[:], bprojc[:], xT[:, sl], op0=OP.add, op1=OP.add
        )
        ln_stats_tile(xf2, t)
        if t == 3:
            ln_finish_half(0)  # MLP tiles 0-3 unblocked before proj 4-7 done
    ln_finish_half(1)

    # ---------------- phase 5: MLP + residual ----------------
    for t in range(NT):
        sl = slice(TT * t, TT * t + TT)
        hn = h1p.tile([128, TT], F32R, tag="hn")
        nc.vector.tensor_mul(hn[:], xc4[:, sl], var4[:, sl])
        o2 = ps_lin([128, 512], "o2")
        for half in range(2):
            hp = ps_pair("01" if half == 0 else "23", "hp")
            gel = gelp.tile([128, 2 * TT], BF16, tag="gel", bufs=2)
            for hh2 in range(2):
                hh = 2 * half + hh2
                mm(
                    hp[:, TT * hh2 : TT * hh2 + TT],
                    W1gT[:, 128 * hh : 128 * hh + 128],
                    hn[:],
                    start=True, stop=True,
                )
                nc.scalar.activation(
                    gel[:, TT * hh2 : TT * hh2 + TT],
                    hp[:, TT * hh2 : TT * hh2 + TT],
                    AF.Gelu,
                    bias=gbcols[:, hh : hh + 1],
                )
            for hh2 in range(2):
                hh = 2 * half + hh2
                mm(
                    o2[:],
                    W2T[:, 128 * hh : 128 * hh + 128],
                    gel[:, TT * hh2 : TT * hh2 + TT],
                    start=(hh == 0), stop=(hh == 3),
                )
        ot = outp.tile([128, TT], F32, tag="ot")
        nc.vector.scalar_tensor_tensor(
            ot[:], o2[:], b2c[:], xf2[:, sl], op0=OP.add, op1=OP.add
        )
        dma(io["out"][:, sl], ot[:])


_NC_CACHE = {}


def build_nc():
    key = "nc"
    if key in _NC_CACHE:
        return _NC_CACHE[key]
    nc = bacc.Bacc("TRN2", target_bir_lowering=False, debug=False)
    io = {}
    for name in INPUT_NAMES:
        io[name] = nc.dram_tensor(
            name, INPUT_SHAPES[name], F32, kind="ExternalInput"
        ).ap()
    io["out"] = nc.dram_tensor("out", [C, L], F32, kind="ExternalOutput").ap()
    with tile.TileContext(nc) as tc:
        with ExitStack() as ctx:
            emit(ctx, tc, io)
    nc.compile()
    _NC_CACHE[key] = nc
    return nc


def make_in_maps(inputs):
    in_maps = []
    for b in range(B):
        m = {
            "x": np.ascontiguousarray(
                inputs["x"][b].reshape(C, L).astype(np.float32)
            ),
            "z": np.ascontiguousarray(inputs["z"][b].astype(np.float32)),
        }
        for name in INPUT_NAMES:
            if name in ("x", "z"):
                continue
            m[name] = np.ascontiguousarray(np.asarray(inputs[name], np.float32))
        in_maps.append(m)
    return in_maps


def kernel(**inputs):
    nc = build_nc()
    in_maps = make_in_maps(inputs)
    res = bass_utils.run_bass_kernel_spmd(nc, in_maps, list(range(B)))
    out = np.stack([res.results[b]["out"].reshape(C, H, W) for b in range(B)])
    return out.astype(np.float32)


if __name__ == "__main__":
    # CoreSim numerics check of core 0 against the reference (dev only).
    import sys

    sys.path.insert(0, "/root/problem")
    import reference

    from concourse.bass_interp import CoreSim

    # CoreSim has no Gelu; patch it (HW has a native erf-gelu table).
    import scipy.special
    from concourse import bass_interp

    _orig_act = bass_interp.InstructionExecutor.visit_InstActivation

    def _patched_act(self, instruction, *, reg_snapshot=None):
        if instruction.func == mybir.ActivationFunctionType.Gelu:
            instruction.func = mybir.ActivationFunctionType.Identity
            try:
                _orig_act(self, instruction, reg_snapshot=reg_snapshot)
            finally:
                instruction.func = mybir.ActivationFunctionType.Gelu
            ov = self.view_ap(
                instruction.outs[0],
                bass_interp.Direction.WRITE,
                instruction,
                reg_snapshot=reg_snapshot,
            )
            x = ov.astype(np.float64)
            ov[:] = (
                x * 0.5 * (1.0 + scipy.special.erf(x / np.sqrt(2.0)))
            ).astype(np.float32)
            return
        return _orig_act(self, instruction, reg_snapshot=reg_snapshot)

    bass_interp.InstructionExecutor.visit_InstActivation = _patched_act

    inputs = {k: np.asarray(v) for k, v in reference.setup_inputs().items()}
    expected = np.asarray(reference.reference(**inputs))

    nc = build_nc()
    print("built+compiled", flush=True)
    sim = CoreSim(nc, require_finite=True, require_nnan=True)
    m = make_in_maps(inputs)[0]
    for k, v in m.items():
        sim.tensor(k)[:] = v
    sim.simulate(check_with_hw=False)
    got = sim.tensor("out").reshape(C, H, W)
    exp0 = expected[0]
    err = np.abs(got - exp0)
    denom = np.abs(exp0).max()
    print("absmax err:", err.max(), "rel:", err.max() / denom)
    print(
        "rms rel:",
        np.sqrt(((got - exp0) ** 2).mean()) / np.sqrt((exp0**2).mean()),
    )
